# revision 15
# baseline (speedup 1.0000x reference)
"""Trainium2 Bass kernel for nn_Propagation (gnn_message_passing).

reference:
    q = val @ Wq; k = val @ Wk            # [B,N,R]
    scores = q @ k.T                      # [B,N,N]  (never materialized here)
    edges = scores / (1 + |scores|)
    delta_state = edges @ state           # [B,N]
    delta_val   = edges @ val             # [B,N,D]

Sharding: 8 cores; cores 0-3 own batch 0 row-blocks of 1024, cores 4-7 batch 1.
Per core, flash-attention style over source tiles m (32 x 128):
    scoresT[m,n]  = kT[:,m].T @ qT[:,nwin]      (PE, f32r, K=64, ap=512)
    edgesT        = softsign(scoresT)            (single fused custom DVE op)
    dv[n,0:258]  += edgesT[:,nt].T @ valA[m]     (PE, f32r, ap=258; col 256 = state
                                                  -> delta_state falls out of the
                                                  same matmul)
All matmuls run as float32r (full PE rate, ~13-bit mantissa). The softsign is
ONE 8-stage DVE pass: |s| -> d=|s|+1 -> n=bitcast(~d) -> m=d*n in [-4.5,-4] ->
r=n*(A+B*m) ~ 1/d (minimax A,B, ~1.7e-3) -> e=s*r.
"""
import sys

for _p in ("/opt/trn_rl_repo",):
    if _p not in sys.path:
        sys.path.insert(0, _p)

import numpy as np

import concourse.bacc as bacc
import concourse.tile as tile
from concourse import mybir
from concourse.bass_utils import run_bass_kernel_spmd
from concourse.dve_spec import Spec, Bin, Src0, C0, C1, One, lower, _has_src1
from concourse.dve_uop import AluOp, DveOpSpec
from concourse import dve_ops
from concourse.dve_ops import DveOp

F32 = mybir.dt.float32
F32R = mybir.dt.float32r
BF16 = mybir.dt.bfloat16

B, N, D, R = 2, 4096, 256, 64
N_CORES = 8
ROWS = (B * N) // N_CORES          # 1024 target rows per core
NWIN = 512                         # n-window (scores free dim)
MT = N // 128                      # 32 source tiles
DA = D + 2                         # 258: val + state col + pad (f32r needs even)

# minimax constants for r = n*(A + B*m) ~= 1/d over m in [-4.5, -4]
_a, _b = -4.5, -4.0
SS_B = float(-2.0 / (_a * _b + (_a + _b) ** 2 / 4.0))
SS_A = float(-SS_B * (_a + _b))


def _register_softsign():
    name = "SOFTSIGN_FUSED_ANT"
    if name in dve_ops._SUB_OPCODE_FOR_NAME:
        return next(op for op in dve_ops.OPS if op.name == name)
    a = Bin(AluOp.ABSOLUTE_VALUE, Src0, Src0)
    d = a + One
    n = Bin(AluOp.BITWISE_NOT, d, d)
    m = d * n
    r = n * ((m * C1) + C0)
    body = Src0 * r

    def ref(in0, in1, s0, s1, imm2):
        aa = np.abs(in0).astype(np.float32)
        dd = (aa + np.float32(1)).astype(np.float32)
        nn = (~dd.view(np.int32)).view(np.float32)
        mm = (dd * nn).astype(np.float32)
        rr = (nn * ((mm * np.float32(s1)) + np.float32(s0))).astype(np.float32)
        return (in0 * rr).astype(np.float32)

    spec = Spec(body=body, reference=ref)
    row = dve_ops._CUSTOM_DVE_ROW_BASE + len(dve_ops.OPS)
    assert row < 0x20
    dve_ops._SUB_OPCODE_FOR_NAME[name] = row
    shas = {}
    for ver in ("v3", "v4"):
        shas[ver] = DveOpSpec(
            name=name, opcode=row, uops=lower(spec, ver=ver), rd1_en=_has_src1(spec)
        ).sha(ver)
    op = DveOp(name, spec, subdim=False, uops_sha=shas)
    dve_ops.OPS.append(op)
    dve_ops.CUSTOM_DVE_SPECS[name] = spec
    return op


_NC_CACHE = {}


def _build():
    if "nc" in _NC_CACHE:
        return _NC_CACHE["nc"]
    softsign = _register_softsign()
    nc = bacc.Bacc("TRN2", target_bir_lowering=False, debug=False)

    valb = nc.dram_tensor("valb", [N, DA], BF16, kind="ExternalInput")
    valT = nc.dram_tensor("valT", [D, N], F32R, kind="ExternalInput")
    wqk = nc.dram_tensor("wqk", [D, 2 * R], F32R, kind="ExternalInput")
    out = nc.dram_tensor("out", [ROWS, DA], F32, kind="ExternalOutput")

    with tile.TileContext(nc) as tc:
        with (
            tc.tile_pool(name="w", bufs=1) as wpool,
            tc.tile_pool(name="vT", bufs=1) as vTpool,
            tc.tile_pool(name="vA", bufs=1) as vApool,
            tc.tile_pool(name="kq", bufs=1) as kqpool,
            tc.tile_pool(name="edges", bufs=3) as epool,
            tc.tile_pool(name="ostage", bufs=4) as opool,
            tc.tile_pool(name="ps_s", bufs=2, space="PSUM") as ps_spool,
            tc.tile_pool(name="ps_dv", bufs=4, space="PSUM") as ps_dvpool,
        ):
            # --- inputs on ONE sync HWDGE queue, in consumption order:
            #     vT c0, wqk, vA g0, vT c1, vA g1, ... (deterministic arrival)
            CH = [(i * 512, (i + 1) * 512) for i in range(8)]
            vTt, vAg = [], []

            def emit_vT_dma(c):
                c0, c1 = CH[c]
                w_ = c1 - c0
                t = vTpool.tile([128, 2 * w_], F32R, name=f"vT{c}")
                nc.sync.dma_start(
                    t[:].rearrange("p (dc n) -> p dc n", n=w_),
                    valT[:, c0:c1].rearrange("(dc p) n -> p dc n", p=128),
                )
                vTt.append(t)

            def emit_vA_dma(g):
                t = vApool.tile([128, 4 * DA], BF16, name=f"vA{g}", tag=f"vA{g}")
                nc.sync.dma_start(
                    t[:].rearrange("p (t c) -> p t c", c=DA),
                    valb[g * 512:(g + 1) * 512, :].rearrange(
                        "(t p) c -> p t c", p=128
                    ),
                )
                vAg.append(t)

            # HAM warm-up: dependency-free matmuls on a zeroed scratch tile
            # fill the initial DMA-wait so the PE clock gate is released
            # (K=8/8, 2.4 GHz) before the first real matmul arrives.
            warm = wpool.tile([128, NWIN], F32R, name="warm")
            nc.gpsimd.memset(warm[:].bitcast(F32), 0.0)
            ps_warm = ps_dvpool.tile([128, DA], F32, name="ps_warm", tag="dv")
            for _ in range(32):
                nc.tensor.matmul(ps_warm[:], warm[:, 0:128], warm[:, 0:DA])

            wqk_sb = wpool.tile([128, 2 * 2 * R], F32R, name="wqk_sb")
            nc.sync.dma_start(
                wqk_sb[:].rearrange("p (dc r) -> p dc r", r=2 * R),
                wqk.rearrange("(dc p) r -> p dc r", p=128),
            )
            vT0 = []
            for dc in range(2):
                t = vTpool.tile([128, NWIN], F32R, name=f"vT0d{dc}")
                nc.sync.dma_start(t[:], valT[dc * 128:(dc + 1) * 128, 0:NWIN])
                vT0.append(t)
            vTt.append(vT0)
            emit_vA_dma(0)
            for c in range(1, 8):
                emit_vT_dma(c)
                emit_vA_dma(c)

            def wq_ap(dc):
                return wqk_sb[:, dc * 2 * R:dc * 2 * R + R]

            def wk_ap(dc):
                return wqk_sb[:, dc * 2 * R + R:(dc + 1) * 2 * R]

            def vT_ap(dc, mc):
                # [128, 512] slice covering kT chunk mc (cols mc*512..)
                if mc == 0:
                    return vTt[0][dc][:]
                return vTt[mc][:, dc * NWIN:(dc + 1) * NWIN]

            def vA(mt):
                return vAg[mt // 4][:, (mt % 4) * DA:(mt % 4 + 1) * DA]

            # --- qT projection (w=1 emitted lazily before window 1) ---
            qT = [kqpool.tile([64, NWIN], F32R, name=f"qT{w}") for w in range(2)]
            qT_done = [False, False]

            def emit_qT(w):
                if qT_done[w]:
                    return
                qT_done[w] = True
                ps_q = ps_spool.tile([64, NWIN], F32, name="psp", tag="s")
                for dc in range(2):
                    nc.tensor.matmul(
                        ps_q[:],
                        wq_ap(dc),
                        vT_ap(dc, w),
                        start=(dc == 0),
                        stop=(dc == 1),
                    )
                nc.scalar.copy(qT[w][:], ps_q[:])

            emit_qT(0)

            # --- kT projection (emitted just-in-time from the main loop) ---
            kT = [kqpool.tile([64, NWIN], F32R, name=f"kT{mc}") for mc in range(8)]
            kT_done = [False] * 8

            def emit_kT(mc):
                if kT_done[mc]:
                    return
                kT_done[mc] = True
                ps_k = ps_spool.tile([64, NWIN], F32, name="psp", tag="s")
                for dc in range(2):
                    nc.tensor.matmul(
                        ps_k[:],
                        wk_ap(dc),
                        vT_ap(dc, mc),
                        start=(dc == 0),
                        stop=(dc == 1),
                    )
                nc.scalar.copy(kT[mc][:], ps_k[:])

            emit_kT(0)

            # --- main flash loop (software-pipelined: delta(mt) emitted
            #     after scores(mt+1) so PE never stalls on DVE softsign) ---
            def emit_delta(ps_dv, e, ms):
                for h in range(2):
                    mt = 2 * ms + h
                    for nt in range(4):
                        nc.tensor.matmul(
                            ps_dv[nt][:],
                            e[:, h * NWIN + nt * 128:h * NWIN + (nt + 1) * 128],
                            vA(mt),
                            start=(mt == 0),
                            stop=(mt == MT - 1),
                        )

            def emit_out(ps_dv, w):
                for nt in range(4):
                    o = opool.tile([128, DA], F32, name="o", tag="o")
                    nc.scalar.copy(o[:], ps_dv[nt][:])
                    nc.sync.dma_start(
                        out[(w * 4 + nt) * 128:(w * 4 + nt + 1) * 128, :], o[:]
                    )

            pending = None  # (ps_dv, e2, ms) awaiting delta emission
            dvs = {}
            for w in range(2):
                dvs[w] = [
                    ps_dvpool.tile([128, DA], F32, name="dv", tag="dv")
                    for _ in range(4)
                ]
                emit_qT(w)
                for ms in range(MT // 2):  # pairs of source tiles
                    if w == 0 and ms // 2 + 1 < 8:
                        emit_kT(ms // 2 + 1)
                    ps_s = ps_spool.tile([128, 2 * NWIN], F32, name="ps_s", tag="s")
                    for h in range(2):
                        mt = 2 * ms + h
                        nc.tensor.matmul(
                            ps_s[:, h * NWIN:(h + 1) * NWIN],
                            kT[mt // 4][:, (mt % 4) * 128:(mt % 4 + 1) * 128],
                            qT[w][:],
                        )
                    e = epool.tile([128, 2 * NWIN], BF16, name="e", tag="e")
                    nc.vector._custom_dve(
                        softsign, out=e[:], in0=ps_s[:], s0=SS_A, s1=SS_B
                    )
                    if pending is not None:
                        emit_delta(*pending)
                        if pending[2] == MT // 2 - 1:
                            emit_out(dvs[w - 1], w - 1)
                    pending = (dvs[w], e, ms)
            emit_delta(*pending)
            emit_out(dvs[1], 1)

    nc.compile()
    _NC_CACHE["nc"] = nc
    return nc


def _shard(state, val, Wq, Wk):
    import ml_dtypes

    state = np.asarray(state, dtype=np.float32)
    val = np.asarray(val, dtype=np.float32)
    wqk = np.ascontiguousarray(
        np.concatenate(
            [np.asarray(Wq, dtype=np.float32), np.asarray(Wk, dtype=np.float32)],
            axis=1,
        )
    )
    in_maps = []
    for core in range(N_CORES):
        b = core // (N_CORES // B)
        n0 = (core % (N_CORES // B)) * ROWS
        # rotate source rows so this core's target rows sit first; the
        # m-sum is permutation-invariant, so only q/k indexing changes.
        vr = np.roll(val[b], -n0, axis=0)
        sr = np.roll(state[b], -n0)
        vb = np.zeros((N, DA), np.float32)
        vb[:, :D] = vr
        vb[:, D] = sr
        in_maps.append(
            {
                "valb": vb.astype(ml_dtypes.bfloat16),
                "valT": np.ascontiguousarray(vr.T),
                "wqk": wqk,
            }
        )
    return in_maps


def _unshard(results):
    delta_state = np.empty((B, N), np.float32)
    delta_val = np.empty((B, N, D), np.float32)
    for core in range(N_CORES):
        b = core // (N_CORES // B)
        n0 = (core % (N_CORES // B)) * ROWS
        o = results[core]["out"]
        delta_val[b, n0:n0 + ROWS] = o[:, :D]
        delta_state[b, n0:n0 + ROWS] = o[:, D]
    return delta_state, delta_val


def _run(in_maps, **kw):
    nc = _build()
    return run_bass_kernel_spmd(nc, in_maps, core_ids=list(range(N_CORES)), **kw)


def kernel(state, val, Wq, Wk):
    in_maps = _shard(state, val, Wq, Wk)
    res = _run(in_maps)
    return _unshard(res.results)


# revision 16
# speedup vs baseline: 1.0119x; 1.0119x over previous
"""Trainium2 Bass kernel for nn_Propagation (gnn_message_passing).

reference:
    q = val @ Wq; k = val @ Wk            # [B,N,R]
    scores = q @ k.T                      # [B,N,N]  (never materialized here)
    edges = scores / (1 + |scores|)
    delta_state = edges @ state           # [B,N]
    delta_val   = edges @ val             # [B,N,D]

Sharding: 8 cores; cores 0-3 own batch 0 row-blocks of 1024, cores 4-7 batch 1.
Per core, flash-attention style over source tiles m (32 x 128):
    scoresT[m,n]  = kT[:,m].T @ qT[:,nwin]      (PE, f32r, K=64, ap=512)
    edgesT        = softsign(scoresT)            (single fused custom DVE op)
    dv[n,0:258]  += edgesT[:,nt].T @ valA[m]     (PE, f32r, ap=258; col 256 = state
                                                  -> delta_state falls out of the
                                                  same matmul)
All matmuls run as float32r (full PE rate, ~13-bit mantissa). The softsign is
ONE 8-stage DVE pass: |s| -> d=|s|+1 -> n=bitcast(~d) -> m=d*n in [-4.5,-4] ->
r=n*(A+B*m) ~ 1/d (minimax A,B, ~1.7e-3) -> e=s*r.
"""
import sys

for _p in ("/opt/trn_rl_repo",):
    if _p not in sys.path:
        sys.path.insert(0, _p)

import numpy as np

import concourse.bacc as bacc
import concourse.tile as tile
from concourse import mybir
from concourse.bass_utils import run_bass_kernel_spmd
from concourse.dve_spec import Spec, Bin, Src0, C0, C1, One, lower, _has_src1
from concourse.dve_uop import AluOp, DveOpSpec
from concourse import dve_ops
from concourse.dve_ops import DveOp

F32 = mybir.dt.float32
F32R = mybir.dt.float32r
BF16 = mybir.dt.bfloat16

B, N, D, R = 2, 4096, 256, 64
N_CORES = 8
ROWS = (B * N) // N_CORES          # 1024 target rows per core
NWIN = 512                         # n-window (scores free dim)
MT = N // 128                      # 32 source tiles
DA = D + 2                         # 258: val + state col + pad (f32r needs even)

# minimax constants for r = n*(A + B*m) ~= 1/d over m in [-4.5, -4]
_a, _b = -4.5, -4.0
SS_B = float(-2.0 / (_a * _b + (_a + _b) ** 2 / 4.0))
SS_A = float(-SS_B * (_a + _b))


def _register_softsign():
    name = "SOFTSIGN_FUSED_ANT"
    if name in dve_ops._SUB_OPCODE_FOR_NAME:
        return next(op for op in dve_ops.OPS if op.name == name)
    a = Bin(AluOp.ABSOLUTE_VALUE, Src0, Src0)
    d = a + One
    n = Bin(AluOp.BITWISE_NOT, d, d)
    m = d * n
    r = n * ((m * C1) + C0)
    body = Src0 * r

    def ref(in0, in1, s0, s1, imm2):
        aa = np.abs(in0).astype(np.float32)
        dd = (aa + np.float32(1)).astype(np.float32)
        nn = (~dd.view(np.int32)).view(np.float32)
        mm = (dd * nn).astype(np.float32)
        rr = (nn * ((mm * np.float32(s1)) + np.float32(s0))).astype(np.float32)
        return (in0 * rr).astype(np.float32)

    spec = Spec(body=body, reference=ref)
    row = dve_ops._CUSTOM_DVE_ROW_BASE + len(dve_ops.OPS)
    assert row < 0x20
    dve_ops._SUB_OPCODE_FOR_NAME[name] = row
    shas = {}
    for ver in ("v3", "v4"):
        shas[ver] = DveOpSpec(
            name=name, opcode=row, uops=lower(spec, ver=ver), rd1_en=_has_src1(spec)
        ).sha(ver)
    op = DveOp(name, spec, subdim=False, uops_sha=shas)
    dve_ops.OPS.append(op)
    dve_ops.CUSTOM_DVE_SPECS[name] = spec
    return op


_NC_CACHE = {}


def _build():
    if "nc" in _NC_CACHE:
        return _NC_CACHE["nc"]
    softsign = _register_softsign()
    nc = bacc.Bacc("TRN2", target_bir_lowering=False, debug=False)

    valb = nc.dram_tensor("valb", [N, DA], BF16, kind="ExternalInput")
    valT = nc.dram_tensor("valT", [D, N], F32R, kind="ExternalInput")
    wqk = nc.dram_tensor("wqk", [D, 2 * R], F32R, kind="ExternalInput")
    out = nc.dram_tensor("out", [ROWS, DA], F32, kind="ExternalOutput")

    with tile.TileContext(nc) as tc:
        with (
            tc.tile_pool(name="w", bufs=1) as wpool,
            tc.tile_pool(name="vT", bufs=1) as vTpool,
            tc.tile_pool(name="vA", bufs=1) as vApool,
            tc.tile_pool(name="kq", bufs=1) as kqpool,
            tc.tile_pool(name="edges", bufs=3) as epool,
            tc.tile_pool(name="ostage", bufs=4) as opool,
            tc.tile_pool(name="ps_s", bufs=2, space="PSUM") as ps_spool,
            tc.tile_pool(name="ps_dv", bufs=4, space="PSUM") as ps_dvpool,
        ):
            # --- inputs on ONE sync HWDGE queue, in consumption order:
            #     vT c0, wqk, vA g0, vT c1, vA g1, ... (deterministic arrival)
            CH = [(i * 512, (i + 1) * 512) for i in range(8)]
            vTt, vAg = [], []

            def emit_vT_dma(c):
                c0, c1 = CH[c]
                w_ = c1 - c0
                t = vTpool.tile([128, 2 * w_], F32R, name=f"vT{c}")
                nc.sync.dma_start(
                    t[:].rearrange("p (dc n) -> p dc n", n=w_),
                    valT[:, c0:c1].rearrange("(dc p) n -> p dc n", p=128),
                )
                vTt.append(t)

            def emit_vA_dma(g):
                t = vApool.tile([128, 4 * DA], BF16, name=f"vA{g}", tag=f"vA{g}")
                nc.sync.dma_start(
                    t[:].rearrange("p (t c) -> p t c", c=DA),
                    valb[g * 512:(g + 1) * 512, :].rearrange(
                        "(t p) c -> p t c", p=128
                    ),
                )
                vAg.append(t)

            # HAM warm-up: dependency-free matmuls on a zeroed scratch tile
            # fill the initial DMA-wait so the PE clock gate is released
            # (K=8/8, 2.4 GHz) before the first real matmul arrives.
            warm = wpool.tile([128, NWIN], F32R, name="warm")
            nc.gpsimd.memset(warm[:].bitcast(F32), 0.0)
            ps_warm = ps_dvpool.tile([128, DA], F32, name="ps_warm", tag="dv")
            for _ in range(32):
                nc.tensor.matmul(ps_warm[:], warm[:, 0:128], warm[:, 0:DA])

            wqk_sb = wpool.tile([128, 2 * 2 * R], F32R, name="wqk_sb")
            nc.sync.dma_start(
                wqk_sb[:].rearrange("p (dc r) -> p dc r", r=2 * R),
                wqk.rearrange("(dc p) r -> p dc r", p=128),
            )
            vT0 = []
            for dc in range(2):
                t = vTpool.tile([128, NWIN], F32R, name=f"vT0d{dc}")
                nc.sync.dma_start(t[:], valT[dc * 128:(dc + 1) * 128, 0:NWIN])
                vT0.append(t)
            vTt.append(vT0)
            emit_vA_dma(0)
            for c in range(1, 8):
                emit_vT_dma(c)
                emit_vA_dma(c)

            def wq_ap(dc):
                return wqk_sb[:, dc * 2 * R:dc * 2 * R + R]

            def wk_ap(dc):
                return wqk_sb[:, dc * 2 * R + R:(dc + 1) * 2 * R]

            def vT_ap(dc, mc):
                # [128, 512] slice covering kT chunk mc (cols mc*512..)
                if mc == 0:
                    return vTt[0][dc][:]
                return vTt[mc][:, dc * NWIN:(dc + 1) * NWIN]

            def vA(mt):
                return vAg[mt // 4][:, (mt % 4) * DA:(mt % 4 + 1) * DA]

            # --- qT projection (w=1 emitted lazily before window 1) ---
            qT = [kqpool.tile([64, NWIN], F32R, name=f"qT{w}") for w in range(2)]
            qT_done = [False, False]

            def emit_qT(w):
                if qT_done[w]:
                    return
                qT_done[w] = True
                ps_q = ps_spool.tile([64, NWIN], F32, name="psp", tag="s")
                for dc in range(2):
                    nc.tensor.matmul(
                        ps_q[:],
                        wq_ap(dc),
                        vT_ap(dc, w),
                        start=(dc == 0),
                        stop=(dc == 1),
                    )
                nc.scalar.copy(qT[w][:], ps_q[:])

            emit_qT(0)

            # --- kT projection (emitted just-in-time from the main loop) ---
            kT = [kqpool.tile([64, NWIN], F32R, name=f"kT{mc}") for mc in range(8)]
            kT_done = [False] * 8

            def emit_kT(mc):
                if kT_done[mc]:
                    return
                kT_done[mc] = True
                ps_k = ps_spool.tile([64, NWIN], F32, name="psp", tag="s")
                for dc in range(2):
                    nc.tensor.matmul(
                        ps_k[:],
                        wk_ap(dc),
                        vT_ap(dc, mc),
                        start=(dc == 0),
                        stop=(dc == 1),
                    )
                nc.scalar.copy(kT[mc][:], ps_k[:])

            emit_kT(0)

            # --- main flash loop (software-pipelined: delta(mt) emitted
            #     after scores(mt+1) so PE never stalls on DVE softsign) ---
            def emit_delta(ps_dv, e, ms):
                for h in range(2):
                    mt = 2 * ms + h
                    for nt in range(4):
                        nc.tensor.matmul(
                            ps_dv[nt][:],
                            e[:, h * NWIN + nt * 128:h * NWIN + (nt + 1) * 128],
                            vA(mt),
                            start=(mt == 0),
                            stop=(mt == MT - 1),
                        )

            def emit_out(ps_dv, w):
                for nt in range(4):
                    o = opool.tile([128, DA], F32, name="o", tag="o")
                    nc.scalar.copy(o[:], ps_dv[nt][:])
                    nc.sync.dma_start(
                        out[(w * 4 + nt) * 128:(w * 4 + nt + 1) * 128, :], o[:]
                    )

            pending = None  # (ps_dv, e2, ms) awaiting delta emission
            dvs = {}
            for w in range(2):
                dvs[w] = [
                    ps_dvpool.tile([128, DA], F32, name="dv", tag="dv")
                    for _ in range(4)
                ]
                emit_qT(w)
                for ms in range(MT // 2):  # pairs of source tiles
                    if w == 0 and ms // 2 + 1 < 8:
                        emit_kT(ms // 2 + 1)
                    if w == 0 and ms == MT // 2 - 3:
                        emit_qT(1)  # prefetch next window's qT off the boundary
                    ps_s = ps_spool.tile([128, 2 * NWIN], F32, name="ps_s", tag="s")
                    for h in range(2):
                        mt = 2 * ms + h
                        nc.tensor.matmul(
                            ps_s[:, h * NWIN:(h + 1) * NWIN],
                            kT[mt // 4][:, (mt % 4) * 128:(mt % 4 + 1) * 128],
                            qT[w][:],
                        )
                    e = epool.tile([128, 2 * NWIN], BF16, name="e", tag="e")
                    nc.vector._custom_dve(
                        softsign, out=e[:], in0=ps_s[:], s0=SS_A, s1=SS_B
                    )
                    if pending is not None:
                        emit_delta(*pending)
                        if pending[2] == MT // 2 - 1:
                            emit_out(dvs[w - 1], w - 1)
                    pending = (dvs[w], e, ms)
            emit_delta(*pending)
            emit_out(dvs[1], 1)

    nc.compile()
    _NC_CACHE["nc"] = nc
    return nc


def _shard(state, val, Wq, Wk):
    import ml_dtypes

    state = np.asarray(state, dtype=np.float32)
    val = np.asarray(val, dtype=np.float32)
    wqk = np.ascontiguousarray(
        np.concatenate(
            [np.asarray(Wq, dtype=np.float32), np.asarray(Wk, dtype=np.float32)],
            axis=1,
        )
    )
    in_maps = []
    for core in range(N_CORES):
        b = core // (N_CORES // B)
        n0 = (core % (N_CORES // B)) * ROWS
        # rotate source rows so this core's target rows sit first; the
        # m-sum is permutation-invariant, so only q/k indexing changes.
        vr = np.roll(val[b], -n0, axis=0)
        sr = np.roll(state[b], -n0)
        vb = np.zeros((N, DA), np.float32)
        vb[:, :D] = vr
        vb[:, D] = sr
        in_maps.append(
            {
                "valb": vb.astype(ml_dtypes.bfloat16),
                "valT": np.ascontiguousarray(vr.T),
                "wqk": wqk,
            }
        )
    return in_maps


def _unshard(results):
    delta_state = np.empty((B, N), np.float32)
    delta_val = np.empty((B, N, D), np.float32)
    for core in range(N_CORES):
        b = core // (N_CORES // B)
        n0 = (core % (N_CORES // B)) * ROWS
        o = results[core]["out"]
        delta_val[b, n0:n0 + ROWS] = o[:, :D]
        delta_state[b, n0:n0 + ROWS] = o[:, D]
    return delta_state, delta_val


def _run(in_maps, **kw):
    nc = _build()
    return run_bass_kernel_spmd(nc, in_maps, core_ids=list(range(N_CORES)), **kw)


def kernel(state, val, Wq, Wk):
    in_maps = _shard(state, val, Wq, Wk)
    res = _run(in_maps)
    return _unshard(res.results)


# revision 17
# speedup vs baseline: 1.0171x; 1.0051x over previous
"""Trainium2 Bass kernel for nn_Propagation (gnn_message_passing).

reference:
    q = val @ Wq; k = val @ Wk            # [B,N,R]
    scores = q @ k.T                      # [B,N,N]  (never materialized here)
    edges = scores / (1 + |scores|)
    delta_state = edges @ state           # [B,N]
    delta_val   = edges @ val             # [B,N,D]

Sharding: 8 cores; cores 0-3 own batch 0 row-blocks of 1024, cores 4-7 batch 1.
Per core, flash-attention style over source tiles m (32 x 128):
    scoresT[m,n]  = kT[:,m].T @ qT[:,nwin]      (PE, f32r, K=64, ap=512)
    edgesT        = softsign(scoresT)            (single fused custom DVE op)
    dv[n,0:258]  += edgesT[:,nt].T @ valA[m]     (PE, f32r, ap=258; col 256 = state
                                                  -> delta_state falls out of the
                                                  same matmul)
All matmuls run as float32r (full PE rate, ~13-bit mantissa). The softsign is
ONE 8-stage DVE pass: |s| -> d=|s|+1 -> n=bitcast(~d) -> m=d*n in [-4.5,-4] ->
r=n*(A+B*m) ~ 1/d (minimax A,B, ~1.7e-3) -> e=s*r.
"""
import sys

for _p in ("/opt/trn_rl_repo",):
    if _p not in sys.path:
        sys.path.insert(0, _p)

import numpy as np

import concourse.bacc as bacc
import concourse.tile as tile
from concourse import mybir
from concourse.bass_utils import run_bass_kernel_spmd
from concourse.dve_spec import Spec, Bin, Src0, C0, C1, One, lower, _has_src1
from concourse.dve_uop import AluOp, DveOpSpec
from concourse import dve_ops
from concourse.dve_ops import DveOp

F32 = mybir.dt.float32
F32R = mybir.dt.float32r
BF16 = mybir.dt.bfloat16

B, N, D, R = 2, 4096, 256, 64
N_CORES = 8
ROWS = (B * N) // N_CORES          # 1024 target rows per core
NWIN = 512                         # n-window (scores free dim)
MT = N // 128                      # 32 source tiles
DA = D + 2                         # 258: val + state col + pad (f32r needs even)

# minimax constants for r = n*(A + B*m) ~= 1/d over m in [-4.5, -4]
_a, _b = -4.5, -4.0
SS_B = float(-2.0 / (_a * _b + (_a + _b) ** 2 / 4.0))
SS_A = float(-SS_B * (_a + _b))


def _register_softsign():
    name = "SOFTSIGN_FUSED_ANT"
    if name in dve_ops._SUB_OPCODE_FOR_NAME:
        return next(op for op in dve_ops.OPS if op.name == name)
    a = Bin(AluOp.ABSOLUTE_VALUE, Src0, Src0)
    d = a + One
    n = Bin(AluOp.BITWISE_NOT, d, d)
    m = d * n
    r = n * ((m * C1) + C0)
    body = Src0 * r

    def ref(in0, in1, s0, s1, imm2):
        aa = np.abs(in0).astype(np.float32)
        dd = (aa + np.float32(1)).astype(np.float32)
        nn = (~dd.view(np.int32)).view(np.float32)
        mm = (dd * nn).astype(np.float32)
        rr = (nn * ((mm * np.float32(s1)) + np.float32(s0))).astype(np.float32)
        return (in0 * rr).astype(np.float32)

    spec = Spec(body=body, reference=ref)
    row = dve_ops._CUSTOM_DVE_ROW_BASE + len(dve_ops.OPS)
    assert row < 0x20
    dve_ops._SUB_OPCODE_FOR_NAME[name] = row
    shas = {}
    for ver in ("v3", "v4"):
        shas[ver] = DveOpSpec(
            name=name, opcode=row, uops=lower(spec, ver=ver), rd1_en=_has_src1(spec)
        ).sha(ver)
    op = DveOp(name, spec, subdim=False, uops_sha=shas)
    dve_ops.OPS.append(op)
    dve_ops.CUSTOM_DVE_SPECS[name] = spec
    return op


_NC_CACHE = {}


def _build():
    if "nc" in _NC_CACHE:
        return _NC_CACHE["nc"]
    softsign = _register_softsign()
    nc = bacc.Bacc("TRN2", target_bir_lowering=False, debug=False)

    valb = nc.dram_tensor("valb", [N, DA], BF16, kind="ExternalInput")
    valT = nc.dram_tensor("valT", [D, N], F32R, kind="ExternalInput")
    wqk = nc.dram_tensor("wqk", [D, 2 * R], F32R, kind="ExternalInput")
    out = nc.dram_tensor("out", [ROWS, DA], F32, kind="ExternalOutput")

    with tile.TileContext(nc) as tc:
        with (
            tc.tile_pool(name="w", bufs=1) as wpool,
            tc.tile_pool(name="vT", bufs=1) as vTpool,
            tc.tile_pool(name="vA", bufs=1) as vApool,
            tc.tile_pool(name="kq", bufs=1) as kqpool,
            tc.tile_pool(name="edges", bufs=4) as epool,
            tc.tile_pool(name="ostage", bufs=4) as opool,
            tc.tile_pool(name="ps_s", bufs=2, space="PSUM") as ps_spool,
            tc.tile_pool(name="ps_dv", bufs=4, space="PSUM") as ps_dvpool,
        ):
            # --- inputs on ONE sync HWDGE queue, in consumption order:
            #     vT c0, wqk, vA g0, vT c1, vA g1, ... (deterministic arrival)
            CH = [(i * 512, (i + 1) * 512) for i in range(8)]
            vTt, vAg = [], []

            def emit_vT_dma(c):
                c0, c1 = CH[c]
                w_ = c1 - c0
                t = vTpool.tile([128, 2 * w_], F32R, name=f"vT{c}")
                nc.sync.dma_start(
                    t[:].rearrange("p (dc n) -> p dc n", n=w_),
                    valT[:, c0:c1].rearrange("(dc p) n -> p dc n", p=128),
                )
                vTt.append(t)

            def emit_vA_dma(g):
                t = vApool.tile([128, 4 * DA], BF16, name=f"vA{g}", tag=f"vA{g}")
                nc.sync.dma_start(
                    t[:].rearrange("p (t c) -> p t c", c=DA),
                    valb[g * 512:(g + 1) * 512, :].rearrange(
                        "(t p) c -> p t c", p=128
                    ),
                )
                vAg.append(t)

            # HAM warm-up: dependency-free matmuls on a zeroed scratch tile
            # fill the initial DMA-wait so the PE clock gate is released
            # (K=8/8, 2.4 GHz) before the first real matmul arrives.
            warm = wpool.tile([128, NWIN], F32R, name="warm")
            nc.gpsimd.memset(warm[:].bitcast(F32), 0.0)
            ps_warm = ps_dvpool.tile([128, DA], F32, name="ps_warm", tag="dv")
            for _ in range(32):
                nc.tensor.matmul(ps_warm[:], warm[:, 0:128], warm[:, 0:DA])

            wqk_sb = wpool.tile([128, 2 * 2 * R], F32R, name="wqk_sb")
            nc.sync.dma_start(
                wqk_sb[:].rearrange("p (dc r) -> p dc r", r=2 * R),
                wqk.rearrange("(dc p) r -> p dc r", p=128),
            )
            vT0 = []
            for dc in range(2):
                t = vTpool.tile([128, NWIN], F32R, name=f"vT0d{dc}")
                nc.sync.dma_start(t[:], valT[dc * 128:(dc + 1) * 128, 0:NWIN])
                vT0.append(t)
            vTt.append(vT0)
            emit_vA_dma(0)
            for c in range(1, 8):
                emit_vT_dma(c)
                emit_vA_dma(c)

            def wq_ap(dc):
                return wqk_sb[:, dc * 2 * R:dc * 2 * R + R]

            def wk_ap(dc):
                return wqk_sb[:, dc * 2 * R + R:(dc + 1) * 2 * R]

            def vT_ap(dc, mc):
                # [128, 512] slice covering kT chunk mc (cols mc*512..)
                if mc == 0:
                    return vTt[0][dc][:]
                return vTt[mc][:, dc * NWIN:(dc + 1) * NWIN]

            def vA(mt):
                return vAg[mt // 4][:, (mt % 4) * DA:(mt % 4 + 1) * DA]

            # --- qT projection (w=1 emitted lazily before window 1) ---
            qT = [kqpool.tile([64, NWIN], F32R, name=f"qT{w}") for w in range(2)]
            qT_done = [False, False]

            def emit_qT(w):
                if qT_done[w]:
                    return
                qT_done[w] = True
                ps_q = ps_spool.tile([64, NWIN], F32, name="psp", tag="s")
                for dc in range(2):
                    nc.tensor.matmul(
                        ps_q[:],
                        wq_ap(dc),
                        vT_ap(dc, w),
                        start=(dc == 0),
                        stop=(dc == 1),
                    )
                nc.scalar.copy(qT[w][:], ps_q[:])

            emit_qT(0)

            # --- kT projection (emitted just-in-time from the main loop) ---
            kT = [kqpool.tile([64, NWIN], F32R, name=f"kT{mc}") for mc in range(8)]
            kT_done = [False] * 8

            def emit_kT(mc):
                if kT_done[mc]:
                    return
                kT_done[mc] = True
                ps_k = ps_spool.tile([64, NWIN], F32, name="psp", tag="s")
                for dc in range(2):
                    nc.tensor.matmul(
                        ps_k[:],
                        wk_ap(dc),
                        vT_ap(dc, mc),
                        start=(dc == 0),
                        stop=(dc == 1),
                    )
                nc.scalar.copy(kT[mc][:], ps_k[:])

            emit_kT(0)

            # --- main flash loop (software-pipelined: delta(mt) emitted
            #     after scores(mt+1) so PE never stalls on DVE softsign) ---
            def emit_delta(ps_dv, e, ms):
                # nt-outer: each accumulator bank hits its stop-matmul as
                # early as possible, so tail copies/DMAs overlap the rest
                for nt in range(4):
                    for h in range(2):
                        mt = 2 * ms + h
                        nc.tensor.matmul(
                            ps_dv[nt][:],
                            e[:, h * NWIN + nt * 128:h * NWIN + (nt + 1) * 128],
                            vA(mt),
                            start=(mt == 0),
                            stop=(mt == MT - 1),
                        )

            def emit_out(ps_dv, w):
                for nt in range(4):
                    o = opool.tile([128, DA], F32, name="o", tag="o")
                    nc.scalar.copy(o[:], ps_dv[nt][:])
                    nc.sync.dma_start(
                        out[(w * 4 + nt) * 128:(w * 4 + nt + 1) * 128, :], o[:]
                    )

            pending = None  # (ps_dv, e2, ms) awaiting delta emission
            dvs = {}
            for w in range(2):
                dvs[w] = [
                    ps_dvpool.tile([128, DA], F32, name="dv", tag="dv")
                    for _ in range(4)
                ]
                emit_qT(w)
                for ms in range(MT // 2):  # pairs of source tiles
                    if w == 0 and ms // 2 + 1 < 8:
                        emit_kT(ms // 2 + 1)
                    if w == 0 and ms == MT // 2 - 3:
                        emit_qT(1)  # prefetch next window's qT off the boundary
                    ps_s = ps_spool.tile([128, 2 * NWIN], F32, name="ps_s", tag="s")
                    for h in range(2):
                        mt = 2 * ms + h
                        nc.tensor.matmul(
                            ps_s[:, h * NWIN:(h + 1) * NWIN],
                            kT[mt // 4][:, (mt % 4) * 128:(mt % 4 + 1) * 128],
                            qT[w][:],
                        )
                    e = epool.tile([128, 2 * NWIN], BF16, name="e", tag="e")
                    nc.vector._custom_dve(
                        softsign, out=e[:], in0=ps_s[:], s0=SS_A, s1=SS_B
                    )
                    if pending is not None:
                        emit_delta(*pending)
                        if pending[2] == MT // 2 - 1:
                            emit_out(dvs[w - 1], w - 1)
                    pending = (dvs[w], e, ms)
            emit_delta(*pending)
            emit_out(dvs[1], 1)

    nc.compile()
    _NC_CACHE["nc"] = nc
    return nc


def _shard(state, val, Wq, Wk):
    import ml_dtypes

    state = np.asarray(state, dtype=np.float32)
    val = np.asarray(val, dtype=np.float32)
    wqk = np.ascontiguousarray(
        np.concatenate(
            [np.asarray(Wq, dtype=np.float32), np.asarray(Wk, dtype=np.float32)],
            axis=1,
        )
    )
    in_maps = []
    for core in range(N_CORES):
        b = core // (N_CORES // B)
        n0 = (core % (N_CORES // B)) * ROWS
        # rotate source rows so this core's target rows sit first; the
        # m-sum is permutation-invariant, so only q/k indexing changes.
        vr = np.roll(val[b], -n0, axis=0)
        sr = np.roll(state[b], -n0)
        vb = np.zeros((N, DA), np.float32)
        vb[:, :D] = vr
        vb[:, D] = sr
        in_maps.append(
            {
                "valb": vb.astype(ml_dtypes.bfloat16),
                "valT": np.ascontiguousarray(vr.T),
                "wqk": wqk,
            }
        )
    return in_maps


def _unshard(results):
    delta_state = np.empty((B, N), np.float32)
    delta_val = np.empty((B, N, D), np.float32)
    for core in range(N_CORES):
        b = core // (N_CORES // B)
        n0 = (core % (N_CORES // B)) * ROWS
        o = results[core]["out"]
        delta_val[b, n0:n0 + ROWS] = o[:, :D]
        delta_state[b, n0:n0 + ROWS] = o[:, D]
    return delta_state, delta_val


def _run(in_maps, **kw):
    nc = _build()
    return run_bass_kernel_spmd(nc, in_maps, core_ids=list(range(N_CORES)), **kw)


def kernel(state, val, Wq, Wk):
    in_maps = _shard(state, val, Wq, Wk)
    res = _run(in_maps)
    return _unshard(res.results)
